# revision 6
# baseline (speedup 1.0000x reference)
"""Trainium2 Bass kernel for nn_MessagePassingConv (GNN message passing).

Strategy (8 NeuronCores, SPMD):
- Nodes sharded by row range: core c owns target nodes [c*62500, (c+1)*62500).
- Edges sharded by TARGET core; x replicated (fp16) to every core so source
  gathers are local. Only a [128,2] BN-stat AllReduce crosses cores.
- Per core, edges sorted by local target, grouped into supertiles of ST=192
  target nodes; each supertile's edges sit in B[S] blocks of 128 lanes.
  Gathers are batched: ONE indirect DMA per ~24-block chunk (amortizes the
  ~1us SWDGE fixed cost that dominated the per-block-gather baseline).
- Scatter via one-hot matmul per block: S^T[f,j] += G[e,f]^T P[e,j],
  P[e,j] = (trel[e] == j), P generated fp16 on DVE (4x mode).
- aggre^T = w_prev^T S_p^T + w_next^T S_n^T + ident^T x^T in PSUM;
  at = relu(aggre + b) written to a PERSISTENT fp16 SBUF buffer (no HBM
  round-trip); BN stats via activation accum_out + AllReduce.
- BN affine folded into GRU weights on device: gks = gk * scol (per-row),
  gate biases += gk_g^T shcol; pass 2 consumes atbuf directly.
- GRU fused in transposed layout, fp16 elementwise, fp16 staged output.
"""
import os
import sys
import types
import contextlib

import numpy as np

import concourse.bass as bass
import concourse.mybir as mybir
import concourse.tile as tile
from concourse.bass import IndirectOffsetOnAxis
from concourse.vector_clock import ScopedClock

F = 128
NCORE = 8
ST = 512
GBUFS = 16
OOB = 1 << 21
BN_EPS = 1e-3
AF = mybir.ActivationFunctionType
ALU = mybir.AluOpType
f32 = mybir.dt.float32
f16 = mybir.dt.float16
i32 = mybir.dt.int32

# ---------------------------------------------------------------- patches

_MAXW = 1


def _patched_drain_and_barrier(self, tick_clock, wait_clock):
    nc = self.nc
    drain_inst = nc.sync.drain()
    wait_clock.add_sem_waits(
        drain_inst.ins, ScopedClock({None: tick_clock.global_clock})
    )
    si = drain_inst.ins.sync_info
    waits = list(si.on_wait) if si and si.on_wait else []
    if len(waits) > _MAXW:
        drain_inst.ins.sync_info = mybir.SyncInfo(
            on_wait=waits[:_MAXW], on_update=list(si.on_update or []))
        for i in range(_MAXW, len(waits), _MAXW):
            extra = nc.sync.drain()
            extra.ins.sync_info = mybir.SyncInfo(
                on_wait=waits[i:i + _MAXW], on_update=[])
    nc.all_engine_barrier()
    assert self.sems is not None
    popped = nc._tile_sem_poison_stack.pop()
    assert popped is self._sem_poison
    nc.clear_and_free_semaphores(list(self.sems.allocated().values()))
    nc.all_engine_barrier()


tile.TileContext._drain_and_barrier = _patched_drain_and_barrier

_split_n = [0]


def _split_excess_waits(nc, cap=1):
    """Walrus codegen caps sync-waits per instruction; move excess onto
    preceding same-engine InstNoOps."""
    for f in nc.m.functions:
        for blk in f.blocks:
            insts = blk.instructions
            if not any(i.sync_info and i.sync_info.on_wait
                       and len(i.sync_info.on_wait) > cap for i in insts):
                continue
            new = []
            for inst in insts:
                si = inst.sync_info
                waits = list(si.on_wait) if si and si.on_wait else []
                if len(waits) > cap:
                    keep, excess = waits[:cap], waits[cap:]
                    for j in range(0, len(excess), cap):
                        _split_n[0] += 1
                        nop = mybir.InstNoOp(
                            name=f"waitsplit-{_split_n[0]}",
                            ins=[], outs=[], engine=inst.engine)
                        nop.sync_info = mybir.SyncInfo(
                            on_wait=excess[j:j + cap], on_update=[])
                        new.append(nop)
                    inst.sync_info = mybir.SyncInfo(
                        on_wait=keep, on_update=list(si.on_update or []))
                new.append(inst)
            blk.instructions = new


def _install_ntff_hook():
    """Provide antenv.axon_hooks (missing in image) so trace=True works."""
    import ctypes
    if "antenv.axon_hooks" in sys.modules:
        return
    try:
        lib = ctypes.CDLL("/opt/axon/libaxon_pjrt.so")
    except OSError:
        return
    if not hasattr(lib, "axon_start_nrt_profile"):
        return
    lib.axon_start_nrt_profile.argtypes = [
        ctypes.POINTER(ctypes.c_int64), ctypes.c_size_t]
    lib.axon_start_nrt_profile.restype = ctypes.c_int64
    lib.axon_stop_nrt_profile.argtypes = [ctypes.c_char_p]
    lib.axon_stop_nrt_profile.restype = ctypes.c_int64

    @contextlib.contextmanager
    def _hook(output_dir, device_ids):
        import jax
        jax.devices()
        if device_ids:
            ids = (ctypes.c_int64 * len(device_ids))(*device_ids)
            rc = lib.axon_start_nrt_profile(ids, len(device_ids))
        else:
            rc = lib.axon_start_nrt_profile(None, 0)
        if rc != 0:
            raise RuntimeError(f"axon_start_nrt_profile rc={rc}")
        try:
            yield
        finally:
            n = lib.axon_stop_nrt_profile(str(output_dir).encode())
            print(f"profile: {n} file(s) written to {output_dir}")

    mod = types.ModuleType("antenv.axon_hooks")
    mod.get_axon_ntff_profile_hook = lambda: _hook
    mod.set_axon_ntff_profile_hook = lambda h: None
    import antenv
    sys.modules["antenv.axon_hooks"] = mod
    antenv.axon_hooks = mod
    from concourse import bass_utils
    bass_utils.upload_artifacts = lambda tmpdir: tmpdir


# ---------------------------------------------------------------- host prep

def _prep_edges(pairs, n_nodes, ncn, nst):
    """Supergroup block schedule: edges grouped by GW-node supergroup, packed
    into blocks of 128 lanes (block count = max over cores). A block's lanes
    may span adjacent 512-wide sub-supertiles; each spanned sub gets one
    one-hot matmul (off-sub lanes have trel=-1 -> zero contribution).

    Returns (pairs_by_s, npairs, nblk, idx_cores, trel_cores):
      pairs_by_s: [nst] lists of (block_id, pair_col)
      idx_cores: per core int32 [128, nblk] gather row indices (OOB pads)
      trel_cores: per core f32 [128, npairs] sub-relative targets (-1 pads)
    """
    GW = 8192
    nsg = (ncn + GW - 1) // GW
    t = np.asarray(pairs[:, 0], dtype=np.int64)
    s_ = np.asarray(pairs[:, 1], dtype=np.int64)
    per_core = []
    cnts = np.zeros((NCORE, nsg), dtype=np.int64)
    for c in range(NCORE):
        m = (t >= c * ncn) & (t < (c + 1) * ncn)
        tl = t[m] - c * ncn
        sc = s_[m]
        order = np.argsort(tl, kind="stable")
        tl = tl[order]
        sc = sc[order]
        sgid = tl // GW
        cnts[c] = np.bincount(sgid, minlength=nsg)
        per_core.append((tl, sc, sgid))
    Bg = np.maximum(1, -(-cnts // 128)).max(axis=0)  # [nsg]
    nblk = int(Bg.sum())
    baseg = np.zeros(nsg, dtype=np.int64)
    baseg[1:] = np.cumsum(Bg)[:-1]
    idx_cores = []
    tgl_cores = []
    for c in range(NCORE):
        tl, sc, sgid = per_core[c]
        idx = np.full((nblk * 128,), OOB, dtype=np.int32)
        tgl = np.full((nblk * 128,), -1, dtype=np.int64)
        estart = np.zeros(nsg + 1, dtype=np.int64)
        np.cumsum(np.bincount(sgid, minlength=nsg), out=estart[1:])
        for g in range(nsg):
            e0, e1 = estart[g], estart[g + 1]
            n = e1 - e0
            if n == 0:
                continue
            lanes = baseg[g] * 128 + np.arange(n)
            idx[lanes] = sc[e0:e1]
            tgl[lanes] = tl[e0:e1]
        idx_cores.append(np.ascontiguousarray(
            idx.reshape(nblk, 128).T))
        tgl_cores.append(tgl.reshape(nblk, 128))
    # sub-supertile span of each block (min/max over cores)
    s_lo = np.full(nblk, 1 << 30, dtype=np.int64)
    s_hi = np.full(nblk, -1, dtype=np.int64)
    for c in range(NCORE):
        tgl = tgl_cores[c]
        valid = tgl >= 0
        anyv = valid.any(axis=1)
        tmin = np.where(valid, tgl, 1 << 40).min(axis=1) // ST
        tmax = np.where(valid, tgl, -1).max(axis=1) // ST
        s_lo[anyv] = np.minimum(s_lo[anyv], tmin[anyv])
        s_hi[anyv] = np.maximum(s_hi[anyv], tmax[anyv])
    # empty blocks (possible only if a supergroup has zero edges everywhere)
    none = s_hi < 0
    gid = np.repeat(np.arange(nsg), Bg.astype(np.int64))
    s_lo[none] = (gid[none] * GW) // ST
    s_hi[none] = s_lo[none]
    # pair list, grouped by sub-supertile
    pairs_by_s = [[] for _ in range(nst)]
    npairs = 0
    pair_block = []
    pair_sub = []
    for j in range(nblk):
        for sidx in range(int(s_lo[j]), int(s_hi[j]) + 1):
            pairs_by_s[sidx].append((j, npairs))
            pair_block.append(j)
            pair_sub.append(sidx)
            npairs += 1
    # per-core trel columns for each pair
    trel_cores = []
    for c in range(NCORE):
        tgl = tgl_cores[c]
        tr = np.full((npairs, 128), -1.0, dtype=np.float32)
        for p in range(npairs):
            j = pair_block[p]
            sidx = pair_sub[p]
            row = tgl[j]
            m = (row >= 0) & (row // ST == sidx)
            tr[p, m] = (row[m] - sidx * ST).astype(np.float32)
        trel_cores.append(np.ascontiguousarray(tr.T))
    return pairs_by_s, npairs, nblk, idx_cores, trel_cores


# ---------------------------------------------------------------- program

def _build_program(n_nodes, pbs_p, npr_p, nblkp, pbs_n, npr_n, nblkn, widths):
    ncn = n_nodes // NCORE
    nst = len(widths)
    nc = bass.Bass()
    x_d = nc.dram_tensor("x16", [n_nodes, F], f16, kind="ExternalInput")
    xt_d = nc.dram_tensor("xtb", [F, ncn], f16, kind="ExternalInput")
    idxp_d = nc.dram_tensor("idx_prev", [128, nblkp], i32, kind="ExternalInput")
    idxn_d = nc.dram_tensor("idx_next", [128, nblkn], i32, kind="ExternalInput")
    trp_d = nc.dram_tensor("trel_prev", [128, npr_p], f32, kind="ExternalInput")
    trn_d = nc.dram_tensor("trel_next", [128, npr_n], f32, kind="ExternalInput")
    wp_d = nc.dram_tensor("w_prev", [F, F], f16, kind="ExternalInput")
    wn_d = nc.dram_tensor("w_next", [F, F], f16, kind="ExternalInput")
    gk_d = nc.dram_tensor("gru_kernel", [F, 3 * F], f16, kind="ExternalInput")
    gr_d = nc.dram_tensor("gru_rec", [F, 3 * F], f16, kind="ExternalInput")
    jt_d = nc.dram_tensor("jtile", [128, ST], f16, kind="ExternalInput")
    id_d = nc.dram_tensor("ident", [F, F], f16, kind="ExternalInput")
    ms_d = nc.dram_tensor("misc", [128, 8], f32, kind="ExternalInput")
    out_d = nc.dram_tensor("outT", [F, ncn], f16, kind="ExternalOutput")
    sin_d = nc.dram_tensor("stats_in", [128, 2], f32)
    sout_d = nc.dram_tensor("stats_out", [128, 2], f32, addr_space="Shared")

    offs = np.concatenate([[0], np.cumsum(widths)])

    with tile.TileContext(nc) as tc:
        with (
            tc.tile_pool(name="const", bufs=1) as cp,
            tc.tile_pool(name="stats", bufs=1) as stp,
        ):
            idxp = cp.tile([128, nblkp], i32)
            idxn = cp.tile([128, nblkn], i32)
            trp = cp.tile([128, npr_p], f32)
            trn = cp.tile([128, npr_n], f32)
            wp = cp.tile([F, F], f16)
            wn = cp.tile([F, F], f16)
            ident = cp.tile([F, F], f16)
            gk = cp.tile([F, 3 * F], f16)
            gr = cp.tile([F, 3 * F], f16)
            jt = cp.tile([128, ST], f16)
            ms = cp.tile([128, 8], f32)
            for sb, d in [(idxp, idxp_d), (idxn, idxn_d), (trp, trp_d),
                          (trn, trn_d), (wp, wp_d), (wn, wn_d),
                          (ident, id_d), (gk, gk_d), (gr, gr_d),
                          (jt, jt_d), (ms, ms_d)]:
                nc.sync.dma_start(sb[:], d[:])
            atbuf = stp.tile([128, ncn], f16)
            st_sum = stp.tile([128, nst], f32)
            st_sq = stp.tile([128, nst], f32)
            breg = nc.gpsimd.to_reg(n_nodes - 1)

            dirs = [
                (idxp, trp, pbs_p),
                (idxn, trn, pbs_n),
            ]
            with (
                tc.tile_pool(name="gat", bufs=GBUFS) as gp,
                tc.tile_pool(name="pone", bufs=6) as pp,
                tc.tile_pool(name="ssb", bufs=4) as sp,
                tc.tile_pool(name="io1", bufs=4) as iop,
                tc.tile_pool(name="psum1", bufs=2, space="PSUM") as ps1,
            ):
                ginit = [0, 0]
                gmap = [{}, {}]
                for S in range(nst):
                    w = int(widths[S])
                    off = int(offs[S])
                    per_set = []
                    for d in range(2):
                        idxsb, trelsb, pbs = dirs[d]
                        plist = pbs[S]
                        ssb = sp.tile([128, w], f16, tag=f"ssb{d}")
                        if not plist:
                            nc.vector.memset(ssb[:], 0.0)
                            per_set.append(ssb)
                            continue
                        stq = ps1.tile([128, w], f32, tag=f"stq{d}")
                        for k, (bj, pc) in enumerate(plist):
                            G = gmap[d].get(bj)
                            if G is None:
                                G = gp.tile([128, 128], f16, tag=f"G{d}")
                                if ginit[d] < GBUFS:
                                    ginit[d] += 1
                                    nc.vector.memset(G[:], 0.0)
                                nc.gpsimd.indirect_dma_start(
                                    out=G[:],
                                    out_offset=None,
                                    in_=x_d[:],
                                    in_offset=IndirectOffsetOnAxis(
                                        ap=idxsb[:, bj:bj + 1], axis=0),
                                    bounds_check=breg,
                                    oob_is_err=False,
                                )
                                gmap[d][bj] = G
                            P = pp.tile([128, w], f16, tag=f"P{d}")
                            nc.any.tensor_scalar(
                                out=P[:], in0=jt[:, :w],
                                scalar1=trelsb[:, pc:pc + 1],
                                scalar2=None, op0=ALU.is_equal)
                            nc.tensor.matmul(
                                stq[:], G[:], P[:],
                                start=(k == 0), stop=(k == len(plist) - 1))
                        nc.any.tensor_copy(ssb[:], stq[:])
                        per_set.append(ssb)
                    sprev, snext = per_set
                    xt = iop.tile([128, w], f16, tag="xt")
                    nc.sync.dma_start(xt[:], xt_d[:, off:off + w])
                    agg = ps1.tile([128, w], f32, tag="agg")
                    nc.tensor.matmul(agg[:], wp[:], sprev[:], start=True, stop=False)
                    nc.tensor.matmul(agg[:], wn[:], snext[:], start=False, stop=False)
                    nc.tensor.matmul(agg[:], ident[:], xt[:], start=False, stop=True)
                    nc.scalar.activation(atbuf[:, off:off + w], agg[:], AF.Relu,
                                         bias=ms[:, 0:1],
                                         accum_out=st_sum[:, S:S + 1])
                    sq = iop.tile([128, w], f16, tag="sq")
                    nc.scalar.activation(sq[:], atbuf[:, off:off + w], AF.Square,
                                         accum_out=st_sq[:, S:S + 1])

            # ---- global BN stats
            red = stp.tile([128, 2], f32)
            nc.vector.reduce_sum(red[:, 0:1], st_sum[:], axis=mybir.AxisListType.X)
            nc.vector.reduce_sum(red[:, 1:2], st_sq[:], axis=mybir.AxisListType.X)
            nc.sync.dma_start(sin_d[:], red[:])
            nc.gpsimd.collective_compute(
                "AllReduce", ALU.add,
                replica_groups=[list(range(NCORE))],
                ins=[sin_d[:]], outs=[sout_d[:]])
            tot = stp.tile([128, 2], f32)
            nc.sync.dma_start(tot[:], sout_d[:])
            mcol = stp.tile([128, 1], f32)
            ecol = stp.tile([128, 1], f32)
            inv_n = 1.0 / float(n_nodes)
            nc.vector.tensor_scalar_mul(mcol[:], tot[:, 0:1], inv_n)
            nc.vector.tensor_scalar_mul(ecol[:], tot[:, 1:2], inv_n)
            msq = stp.tile([128, 1], f32)
            nc.vector.tensor_mul(msq[:], mcol[:], mcol[:])
            var = stp.tile([128, 1], f32)
            nc.vector.tensor_sub(var[:], ecol[:], msq[:])
            sd = stp.tile([128, 1], f32)
            nc.scalar.activation(sd[:], var[:], AF.Sqrt, bias=ms[:, 7:8])
            rstd = stp.tile([128, 1], f32)
            nc.vector.reciprocal(rstd[:], sd[:])
            scol = stp.tile([128, 1], f32)
            nc.vector.tensor_mul(scol[:], rstd[:], ms[:, 5:6])
            tmp = stp.tile([128, 1], f32)
            nc.vector.tensor_mul(tmp[:], mcol[:], scol[:])
            shcol = stp.tile([128, 1], f32)
            nc.vector.tensor_sub(shcol[:], ms[:, 6:7], tmp[:])
            shc16 = stp.tile([128, 1], f16)
            nc.vector.tensor_copy(shc16[:], shcol[:])
            # fold BN scale into the at-side GRU kernel; shift into gate biases
            gks = stp.tile([F, 3 * F], f16)
            nc.vector.tensor_scalar(out=gks[:], in0=gk[:], scalar1=scol[:],
                                    scalar2=None, op0=ALU.mult)
            gbias = []
            with tc.tile_pool(name="psb", bufs=3, space="PSUM") as psb:
                for g in range(3):
                    bcp = psb.tile([128, 1], f32, tag="bc")
                    nc.tensor.matmul(bcp[:], gk[:, g * F:(g + 1) * F], shc16[:],
                                     start=True, stop=True)
                    bg = stp.tile([128, 1], f32, tag=f"bias{g}")
                    nc.vector.tensor_add(bg[:], bcp[:], ms[:, g + 1:g + 2])
                    gbias.append(bg)

            # ---- pass 2: GRU
            with (
                tc.tile_pool(name="p2", bufs=3) as p2,
                tc.tile_pool(name="psum2", bufs=2, space="PSUM") as ps2,
            ):
                for S in range(nst):
                    w = int(widths[S])
                    off = int(offs[S])
                    at_sl = atbuf[:, off:off + w]
                    xt = p2.tile([128, w], f16, tag="xt2")
                    nc.sync.dma_start(xt[:], xt_d[:, off:off + w])
                    zp = ps2.tile([128, w], f32, tag="zp")
                    rp = ps2.tile([128, w], f32, tag="rp")
                    hp = ps2.tile([128, w], f32, tag="hp")
                    gq = ps2.tile([128, w], f32, tag="gq")
                    nc.tensor.matmul(zp[:], gks[:, 0:F], at_sl, start=True, stop=False)
                    nc.tensor.matmul(zp[:], gr[:, 0:F], xt[:], start=False, stop=True)
                    nc.tensor.matmul(rp[:], gks[:, F:2 * F], at_sl, start=True, stop=False)
                    nc.tensor.matmul(rp[:], gr[:, F:2 * F], xt[:], start=False, stop=True)
                    nc.tensor.matmul(hp[:], gks[:, 2 * F:3 * F], at_sl, start=True, stop=True)
                    nc.tensor.matmul(gq[:], gr[:, 2 * F:3 * F], xt[:], start=True, stop=True)
                    z = p2.tile([128, w], f16, tag="z")
                    r = p2.tile([128, w], f16, tag="r")
                    nc.scalar.activation(z[:], zp[:], AF.Sigmoid, bias=gbias[0][:])
                    nc.scalar.activation(r[:], rp[:], AF.Sigmoid, bias=gbias[1][:])
                    rhb = p2.tile([128, w], f16, tag="rhb")
                    nc.any.tensor_scalar(out=rhb[:], in0=gq[:],
                                         scalar1=ms[:, 4:5], scalar2=None,
                                         op0=ALU.add)
                    t1 = p2.tile([128, w], f16, tag="t1")
                    nc.any.tensor_tensor(out=t1[:], in0=r[:], in1=rhb[:],
                                         op=ALU.mult)
                    t2 = p2.tile([128, w], f16, tag="t2")
                    nc.any.tensor_tensor(out=t2[:], in0=hp[:], in1=t1[:],
                                         op=ALU.add)
                    hc = p2.tile([128, w], f16, tag="hc")
                    nc.scalar.activation(hc[:], t2[:], AF.Tanh, bias=gbias[2][:])
                    dd = p2.tile([128, w], f16, tag="dd")
                    nc.any.tensor_tensor(out=dd[:], in0=xt[:], in1=hc[:],
                                         op=ALU.subtract)
                    mm_ = p2.tile([128, w], f16, tag="mm")
                    nc.any.tensor_tensor(out=mm_[:], in0=z[:], in1=dd[:],
                                         op=ALU.mult)
                    oo = p2.tile([128, w], f16, tag="oo")
                    nc.any.tensor_tensor(out=oo[:], in0=hc[:], in1=mm_[:],
                                         op=ALU.add)
                    nc.sync.dma_start(out_d[:, off:off + w], oo[:])

    return nc


# ---------------------------------------------------------------- kernel

def _prepare(inputs):
    x = np.asarray(inputs["x"], dtype=np.float32)
    n_nodes = x.shape[0]
    ncn = n_nodes // NCORE
    nst = (ncn + ST - 1) // ST
    widths = [ST] * (nst - 1) + [ncn - (nst - 1) * ST]

    pbs_p, npr_p, nblkp, idxp_c, trp_c = _prep_edges(
        inputs["pairs_prev"], n_nodes, ncn, nst)
    pbs_n, npr_n, nblkn, idxn_c, trn_c = _prep_edges(
        inputs["pairs_next"], n_nodes, ncn, nst)

    x16 = np.ascontiguousarray(x.astype(np.float16))
    wn = np.ascontiguousarray(np.asarray(inputs["w_next"], np.float16))
    wpv = np.ascontiguousarray(np.asarray(inputs["w_prev"], np.float16))
    gkv = np.ascontiguousarray(np.asarray(inputs["gru_kernel"], np.float16))
    grv = np.ascontiguousarray(
        np.asarray(inputs["gru_rec_kernel"], np.float16))
    gb = np.asarray(inputs["gru_bias"], dtype=np.float32)
    bb = np.asarray(inputs["b"], dtype=np.float32).reshape(-1)
    gamma = np.asarray(inputs["bn_gamma"], dtype=np.float32).reshape(-1)
    beta = np.asarray(inputs["bn_beta"], dtype=np.float32).reshape(-1)

    misc = np.zeros((128, 8), dtype=np.float32)
    misc[:, 0] = bb
    misc[:, 1] = gb[0, 0:F] + gb[1, 0:F]          # z bias
    misc[:, 2] = gb[0, F:2 * F] + gb[1, F:2 * F]  # r bias
    misc[:, 3] = gb[0, 2 * F:3 * F]               # h kernel bias
    misc[:, 4] = gb[1, 2 * F:3 * F]               # h recurrent bias
    misc[:, 5] = gamma
    misc[:, 6] = beta
    misc[:, 7] = BN_EPS
    jt = np.ascontiguousarray(
        np.broadcast_to(np.arange(ST, dtype=np.float16), (128, ST)))
    ident = np.eye(F, dtype=np.float16)

    in_maps = []
    for c in range(NCORE):
        xt_c = np.ascontiguousarray(
            x[c * ncn:(c + 1) * ncn].T.astype(np.float16))
        in_maps.append({
            "x16": x16,
            "xtb": xt_c,
            "idx_prev": idxp_c[c], "idx_next": idxn_c[c],
            "trel_prev": trp_c[c], "trel_next": trn_c[c],
            "w_prev": wpv, "w_next": wn,
            "gru_kernel": gkv, "gru_rec": grv,
            "jtile": jt, "ident": ident, "misc": misc,
        })
    nc = _build_program(n_nodes, pbs_p, npr_p, nblkp, pbs_n, npr_n, nblkn, widths)
    return nc, in_maps, ncn


def kernel(**inputs):
    _install_ntff_hook()
    from concourse.bass_utils import run_bass_kernel_spmd
    nc, in_maps, ncn = _prepare(inputs)
    _split_excess_waits(nc, cap=1)
    trace = bool(int(os.environ.get("KERNEL_TRACE", "0")))
    kw = {}
    if trace:
        kw = dict(trace=True,
                  tmpdir=os.environ.get("KERNEL_TRACE_DIR",
                                        "/tmp/kernel_trace"))
    res = run_bass_kernel_spmd(nc, in_maps, list(range(NCORE)), **kw)
    if trace:
        kernel.last_exec_time_ns = res.exec_time_ns
    out = np.concatenate(
        [res.results[c]["outT"].T.astype(np.float32) for c in range(NCORE)],
        axis=0)
    return out


kernel.last_exec_time_ns = None


# revision 8
# speedup vs baseline: 1.3617x; 1.3617x over previous
"""Trainium2 Bass kernel for nn_MessagePassingConv (GNN message passing).

Strategy (8 NeuronCores, SPMD):
- Nodes sharded by row range: core c owns target nodes [c*62500, (c+1)*62500).
- Edges sharded by TARGET core; x replicated (fp16) to every core so source
  gathers are local. Only a [128,2] BN-stat AllReduce crosses cores.
- Per core, edges sorted by local target, grouped into supertiles of ST=192
  target nodes; each supertile's edges sit in B[S] blocks of 128 lanes.
  Gathers are batched: ONE indirect DMA per ~24-block chunk (amortizes the
  ~1us SWDGE fixed cost that dominated the per-block-gather baseline).
- Scatter via one-hot matmul per block: S^T[f,j] += G[e,f]^T P[e,j],
  P[e,j] = (trel[e] == j), P generated fp16 on DVE (4x mode).
- aggre^T = w_prev^T S_p^T + w_next^T S_n^T + ident^T x^T in PSUM;
  at = relu(aggre + b) written to a PERSISTENT fp16 SBUF buffer (no HBM
  round-trip); BN stats via activation accum_out + AllReduce.
- BN affine folded into GRU weights on device: gks = gk * scol (per-row),
  gate biases += gk_g^T shcol; pass 2 consumes atbuf directly.
- GRU fused in transposed layout, fp16 elementwise, fp16 staged output.
"""
import os
import sys
import types
import contextlib

import numpy as np

import concourse.bass as bass
import concourse.mybir as mybir
import concourse.tile as tile
from concourse.bass import IndirectOffsetOnAxis
from concourse.vector_clock import ScopedClock

F = 128
NCORE = 8
ST = 512
GBUFS = 24
OOB = 1 << 21
BN_EPS = 1e-3
AF = mybir.ActivationFunctionType
ALU = mybir.AluOpType
f32 = mybir.dt.float32
f16 = mybir.dt.float16
f32r = mybir.dt.float32r
i32 = mybir.dt.int32

# ---------------------------------------------------------------- patches

_MAXW = 1


def _patched_drain_and_barrier(self, tick_clock, wait_clock):
    nc = self.nc
    drain_inst = nc.sync.drain()
    wait_clock.add_sem_waits(
        drain_inst.ins, ScopedClock({None: tick_clock.global_clock})
    )
    si = drain_inst.ins.sync_info
    waits = list(si.on_wait) if si and si.on_wait else []
    if len(waits) > _MAXW:
        drain_inst.ins.sync_info = mybir.SyncInfo(
            on_wait=waits[:_MAXW], on_update=list(si.on_update or []))
        for i in range(_MAXW, len(waits), _MAXW):
            extra = nc.sync.drain()
            extra.ins.sync_info = mybir.SyncInfo(
                on_wait=waits[i:i + _MAXW], on_update=[])
    nc.all_engine_barrier()
    assert self.sems is not None
    popped = nc._tile_sem_poison_stack.pop()
    assert popped is self._sem_poison
    nc.clear_and_free_semaphores(list(self.sems.allocated().values()))
    nc.all_engine_barrier()


tile.TileContext._drain_and_barrier = _patched_drain_and_barrier

_split_n = [0]


def _split_excess_waits(nc, cap=1):
    """Walrus codegen caps sync-waits per instruction; move excess onto
    preceding same-engine InstNoOps."""
    for f in nc.m.functions:
        for blk in f.blocks:
            insts = blk.instructions
            if not any(i.sync_info and i.sync_info.on_wait
                       and len(i.sync_info.on_wait) > cap for i in insts):
                continue
            new = []
            for inst in insts:
                si = inst.sync_info
                waits = list(si.on_wait) if si and si.on_wait else []
                if len(waits) > cap:
                    keep, excess = waits[:cap], waits[cap:]
                    for j in range(0, len(excess), cap):
                        _split_n[0] += 1
                        nop = mybir.InstNoOp(
                            name=f"waitsplit-{_split_n[0]}",
                            ins=[], outs=[], engine=inst.engine)
                        nop.sync_info = mybir.SyncInfo(
                            on_wait=excess[j:j + cap], on_update=[])
                        new.append(nop)
                    inst.sync_info = mybir.SyncInfo(
                        on_wait=keep, on_update=list(si.on_update or []))
                new.append(inst)
            blk.instructions = new


def _install_ntff_hook():
    """Provide antenv.axon_hooks (missing in image) so trace=True works."""
    import ctypes
    if "antenv.axon_hooks" in sys.modules:
        return
    try:
        lib = ctypes.CDLL("/opt/axon/libaxon_pjrt.so")
    except OSError:
        return
    if not hasattr(lib, "axon_start_nrt_profile"):
        return
    lib.axon_start_nrt_profile.argtypes = [
        ctypes.POINTER(ctypes.c_int64), ctypes.c_size_t]
    lib.axon_start_nrt_profile.restype = ctypes.c_int64
    lib.axon_stop_nrt_profile.argtypes = [ctypes.c_char_p]
    lib.axon_stop_nrt_profile.restype = ctypes.c_int64

    @contextlib.contextmanager
    def _hook(output_dir, device_ids):
        import jax
        jax.devices()
        if device_ids:
            ids = (ctypes.c_int64 * len(device_ids))(*device_ids)
            rc = lib.axon_start_nrt_profile(ids, len(device_ids))
        else:
            rc = lib.axon_start_nrt_profile(None, 0)
        if rc != 0:
            raise RuntimeError(f"axon_start_nrt_profile rc={rc}")
        try:
            yield
        finally:
            n = lib.axon_stop_nrt_profile(str(output_dir).encode())
            print(f"profile: {n} file(s) written to {output_dir}")

    mod = types.ModuleType("antenv.axon_hooks")
    mod.get_axon_ntff_profile_hook = lambda: _hook
    mod.set_axon_ntff_profile_hook = lambda h: None
    import antenv
    sys.modules["antenv.axon_hooks"] = mod
    antenv.axon_hooks = mod
    from concourse import bass_utils
    bass_utils.upload_artifacts = lambda tmpdir: tmpdir


# ---------------------------------------------------------------- host prep

def _prep_edges(pairs, n_nodes, ncn, nst):
    """Supergroup block schedule: edges grouped by GW-node supergroup, packed
    into blocks of 128 lanes (block count = max over cores). A block's lanes
    may span adjacent 512-wide sub-supertiles; each spanned sub gets one
    one-hot matmul (off-sub lanes have trel=-1 -> zero contribution).

    Returns (pairs_by_s, npairs, nblk, idx_cores, trel_cores):
      pairs_by_s: [nst] lists of (block_id, pair_col)
      idx_cores: per core int32 [128, nblk] gather row indices (OOB pads)
      trel_cores: per core f32 [128, npairs] sub-relative targets (-1 pads)
    """
    GW = 8192
    nsg = (ncn + GW - 1) // GW
    t = np.asarray(pairs[:, 0], dtype=np.int64)
    s_ = np.asarray(pairs[:, 1], dtype=np.int64)
    per_core = []
    cnts = np.zeros((NCORE, nsg), dtype=np.int64)
    for c in range(NCORE):
        m = (t >= c * ncn) & (t < (c + 1) * ncn)
        tl = t[m] - c * ncn
        sc = s_[m]
        order = np.argsort(tl, kind="stable")
        tl = tl[order]
        sc = sc[order]
        sgid = tl // GW
        cnts[c] = np.bincount(sgid, minlength=nsg)
        per_core.append((tl, sc, sgid))
    Bg = np.maximum(1, -(-cnts // 128)).max(axis=0)  # [nsg]
    nblk = int(Bg.sum())
    baseg = np.zeros(nsg, dtype=np.int64)
    baseg[1:] = np.cumsum(Bg)[:-1]
    idx_cores = []
    tgl_cores = []
    for c in range(NCORE):
        tl, sc, sgid = per_core[c]
        idx = np.zeros((nblk * 128,), dtype=np.int32)
        tgl = np.full((nblk * 128,), -1, dtype=np.int64)
        estart = np.zeros(nsg + 1, dtype=np.int64)
        np.cumsum(np.bincount(sgid, minlength=nsg), out=estart[1:])
        for g in range(nsg):
            e0, e1 = estart[g], estart[g + 1]
            n = e1 - e0
            if n == 0:
                continue
            lanes = baseg[g] * 128 + np.arange(n)
            idx[lanes] = sc[e0:e1]
            tgl[lanes] = tl[e0:e1]
        idx_cores.append(np.ascontiguousarray(
            idx.reshape(nblk, 128).T))
        tgl_cores.append(tgl.reshape(nblk, 128))
    # sub-supertile span of each block (min/max over cores)
    s_lo = np.full(nblk, 1 << 30, dtype=np.int64)
    s_hi = np.full(nblk, -1, dtype=np.int64)
    for c in range(NCORE):
        tgl = tgl_cores[c]
        valid = tgl >= 0
        anyv = valid.any(axis=1)
        tmin = np.where(valid, tgl, 1 << 40).min(axis=1) // ST
        tmax = np.where(valid, tgl, -1).max(axis=1) // ST
        s_lo[anyv] = np.minimum(s_lo[anyv], tmin[anyv])
        s_hi[anyv] = np.maximum(s_hi[anyv], tmax[anyv])
    # empty blocks (possible only if a supergroup has zero edges everywhere)
    none = s_hi < 0
    gid = np.repeat(np.arange(nsg), Bg.astype(np.int64))
    s_lo[none] = (gid[none] * GW) // ST
    s_hi[none] = s_lo[none]
    # pair list, grouped by sub-supertile
    pairs_by_s = [[] for _ in range(nst)]
    npairs = 0
    pair_block = []
    pair_sub = []
    for j in range(nblk):
        for sidx in range(int(s_lo[j]), int(s_hi[j]) + 1):
            pairs_by_s[sidx].append((j, npairs))
            pair_block.append(j)
            pair_sub.append(sidx)
            npairs += 1
    # per-core trel columns for each pair
    trel_cores = []
    for c in range(NCORE):
        tgl = tgl_cores[c]
        tr = np.full((npairs, 128), -1.0, dtype=np.float32)
        for p in range(npairs):
            j = pair_block[p]
            sidx = pair_sub[p]
            row = tgl[j]
            m = (row >= 0) & (row // ST == sidx)
            tr[p, m] = (row[m] - sidx * ST).astype(np.float32)
        trel_cores.append(np.ascontiguousarray(tr.T))
    return pairs_by_s, npairs, nblk, idx_cores, trel_cores


# ---------------------------------------------------------------- program

def _build_program(n_nodes, pbs_p, npr_p, nblkp, pbs_n, npr_n, nblkn, widths):
    ncn = n_nodes // NCORE
    nst = len(widths)
    nc = bass.Bass()
    x_d = nc.dram_tensor("x_full", [n_nodes, F], f32r, kind="ExternalInput")
    xt_d = nc.dram_tensor("xtb", [F, ncn], f16, kind="ExternalInput")
    idxp_d = nc.dram_tensor("idx_prev", [128, nblkp], i32, kind="ExternalInput")
    idxn_d = nc.dram_tensor("idx_next", [128, nblkn], i32, kind="ExternalInput")
    trp_d = nc.dram_tensor("trel_prev", [128, npr_p], f32, kind="ExternalInput")
    trn_d = nc.dram_tensor("trel_next", [128, npr_n], f32, kind="ExternalInput")
    wp_d = nc.dram_tensor("w_prev", [F, F], f16, kind="ExternalInput")
    wn_d = nc.dram_tensor("w_next", [F, F], f16, kind="ExternalInput")
    gk_d = nc.dram_tensor("gru_kernel", [F, 3 * F], f16, kind="ExternalInput")
    gr_d = nc.dram_tensor("gru_rec", [F, 3 * F], f16, kind="ExternalInput")
    jt_d = nc.dram_tensor("jtile", [128, ST], f32, kind="ExternalInput")
    id_d = nc.dram_tensor("ident", [F, F], f16, kind="ExternalInput")
    ms_d = nc.dram_tensor("misc", [128, 8], f32, kind="ExternalInput")
    out_d = nc.dram_tensor("outT", [F, ncn], f16, kind="ExternalOutput")
    sin_d = nc.dram_tensor("stats_in", [128, 2], f32)
    sout_d = nc.dram_tensor("stats_out", [128, 2], f32, addr_space="Shared")

    offs = np.concatenate([[0], np.cumsum(widths)])

    with tile.TileContext(nc) as tc:
        with (
            tc.tile_pool(name="const", bufs=1) as cp,
            tc.tile_pool(name="stats", bufs=1) as stp,
        ):
            idxp = cp.tile([128, nblkp], i32)
            idxn = cp.tile([128, nblkn], i32)
            trp = cp.tile([128, npr_p], f32)
            trn = cp.tile([128, npr_n], f32)
            wp = cp.tile([F, F], f16)
            wn = cp.tile([F, F], f16)
            ident = cp.tile([F, F], f16)
            gk = cp.tile([F, 3 * F], f16)
            gr = cp.tile([F, 3 * F], f16)
            jt = cp.tile([128, ST], f32)
            ms = cp.tile([128, 8], f32)
            for sb, d in [(idxp, idxp_d), (idxn, idxn_d), (trp, trp_d),
                          (trn, trn_d), (wp, wp_d), (wn, wn_d),
                          (ident, id_d), (gk, gk_d), (gr, gr_d),
                          (jt, jt_d), (ms, ms_d)]:
                nc.sync.dma_start(sb[:], d[:])
            atbuf = stp.tile([128, ncn], f16)
            st_sum = stp.tile([128, nst], f32)
            st_sq = stp.tile([128, nst], f32)

            dirs = [
                (idxp, trp, pbs_p),
                (idxn, trn, pbs_n),
            ]
            with (
                tc.tile_pool(name="gat", bufs=GBUFS) as gp,
                tc.tile_pool(name="pone", bufs=6) as pp,
                tc.tile_pool(name="ssb", bufs=4) as sp,
                tc.tile_pool(name="io1", bufs=4) as iop,
                tc.tile_pool(name="psum1", bufs=2, space="PSUM") as ps1,
            ):
                gmap = [{}, {}]
                for S in range(nst):
                    w = int(widths[S])
                    off = int(offs[S])
                    per_set = []
                    for d in range(2):
                        idxsb, trelsb, pbs = dirs[d]
                        plist = pbs[S]
                        ssb = sp.tile([128, w], f16, tag=f"ssb{d}")
                        if not plist:
                            nc.vector.memset(ssb[:], 0.0)
                            per_set.append(ssb)
                            continue
                        stq = ps1.tile([128, w], f32, tag=f"stq{d}")
                        for k, (bj, pc) in enumerate(plist):
                            G = gmap[d].get(bj)
                            if G is None:
                                G = gp.tile([128, 128], f32r, tag=f"G{d}")
                                nc.gpsimd.indirect_dma_start(
                                    out=G[:],
                                    out_offset=None,
                                    in_=x_d[:],
                                    in_offset=IndirectOffsetOnAxis(
                                        ap=idxsb[:, bj:bj + 1], axis=0),
                                )
                                gmap[d][bj] = G
                            P = pp.tile([128, w], f32r, tag=f"P{d}")
                            nc.any.tensor_scalar(
                                out=P[:], in0=jt[:, :w],
                                scalar1=trelsb[:, pc:pc + 1],
                                scalar2=None, op0=ALU.is_equal)
                            nc.tensor.matmul(
                                stq[:], G[:], P[:],
                                start=(k == 0), stop=(k == len(plist) - 1))
                        nc.any.tensor_copy(ssb[:], stq[:])
                        per_set.append(ssb)
                    sprev, snext = per_set
                    xt = iop.tile([128, w], f16, tag="xt")
                    nc.sync.dma_start(xt[:], xt_d[:, off:off + w])
                    agg = ps1.tile([128, w], f32, tag="agg")
                    nc.tensor.matmul(agg[:], wp[:], sprev[:], start=True, stop=False)
                    nc.tensor.matmul(agg[:], wn[:], snext[:], start=False, stop=False)
                    nc.tensor.matmul(agg[:], ident[:], xt[:], start=False, stop=True)
                    nc.scalar.activation(atbuf[:, off:off + w], agg[:], AF.Relu,
                                         bias=ms[:, 0:1],
                                         accum_out=st_sum[:, S:S + 1])
                    sq = iop.tile([128, w], f16, tag="sq")
                    nc.scalar.activation(sq[:], atbuf[:, off:off + w], AF.Square,
                                         accum_out=st_sq[:, S:S + 1])

            # ---- global BN stats
            red = stp.tile([128, 2], f32)
            nc.vector.reduce_sum(red[:, 0:1], st_sum[:], axis=mybir.AxisListType.X)
            nc.vector.reduce_sum(red[:, 1:2], st_sq[:], axis=mybir.AxisListType.X)
            nc.sync.dma_start(sin_d[:], red[:])
            nc.gpsimd.collective_compute(
                "AllReduce", ALU.add,
                replica_groups=[list(range(NCORE))],
                ins=[sin_d[:]], outs=[sout_d[:]])
            tot = stp.tile([128, 2], f32)
            nc.sync.dma_start(tot[:], sout_d[:])
            mcol = stp.tile([128, 1], f32)
            ecol = stp.tile([128, 1], f32)
            inv_n = 1.0 / float(n_nodes)
            nc.vector.tensor_scalar_mul(mcol[:], tot[:, 0:1], inv_n)
            nc.vector.tensor_scalar_mul(ecol[:], tot[:, 1:2], inv_n)
            msq = stp.tile([128, 1], f32)
            nc.vector.tensor_mul(msq[:], mcol[:], mcol[:])
            var = stp.tile([128, 1], f32)
            nc.vector.tensor_sub(var[:], ecol[:], msq[:])
            sd = stp.tile([128, 1], f32)
            nc.scalar.activation(sd[:], var[:], AF.Sqrt, bias=ms[:, 7:8])
            rstd = stp.tile([128, 1], f32)
            nc.vector.reciprocal(rstd[:], sd[:])
            scol = stp.tile([128, 1], f32)
            nc.vector.tensor_mul(scol[:], rstd[:], ms[:, 5:6])
            tmp = stp.tile([128, 1], f32)
            nc.vector.tensor_mul(tmp[:], mcol[:], scol[:])
            shcol = stp.tile([128, 1], f32)
            nc.vector.tensor_sub(shcol[:], ms[:, 6:7], tmp[:])
            shc16 = stp.tile([128, 1], f16)
            nc.vector.tensor_copy(shc16[:], shcol[:])
            # fold BN scale into the at-side GRU kernel; shift into gate biases
            gks = stp.tile([F, 3 * F], f16)
            nc.vector.tensor_scalar(out=gks[:], in0=gk[:], scalar1=scol[:],
                                    scalar2=None, op0=ALU.mult)
            gbias = []
            with tc.tile_pool(name="psb", bufs=3, space="PSUM") as psb:
                for g in range(3):
                    bcp = psb.tile([128, 1], f32, tag="bc")
                    nc.tensor.matmul(bcp[:], gk[:, g * F:(g + 1) * F], shc16[:],
                                     start=True, stop=True)
                    bg = stp.tile([128, 1], f32, tag=f"bias{g}")
                    nc.vector.tensor_add(bg[:], bcp[:], ms[:, g + 1:g + 2])
                    gbias.append(bg)

            # ---- pass 2: GRU
            with (
                tc.tile_pool(name="p2", bufs=3) as p2,
                tc.tile_pool(name="psum2", bufs=2, space="PSUM") as ps2,
            ):
                for S in range(nst):
                    w = int(widths[S])
                    off = int(offs[S])
                    at_sl = atbuf[:, off:off + w]
                    xt = p2.tile([128, w], f16, tag="xt2")
                    nc.sync.dma_start(xt[:], xt_d[:, off:off + w])
                    zp = ps2.tile([128, w], f32, tag="zp")
                    rp = ps2.tile([128, w], f32, tag="rp")
                    hp = ps2.tile([128, w], f32, tag="hp")
                    gq = ps2.tile([128, w], f32, tag="gq")
                    nc.tensor.matmul(zp[:], gks[:, 0:F], at_sl, start=True, stop=False)
                    nc.tensor.matmul(zp[:], gr[:, 0:F], xt[:], start=False, stop=True)
                    nc.tensor.matmul(rp[:], gks[:, F:2 * F], at_sl, start=True, stop=False)
                    nc.tensor.matmul(rp[:], gr[:, F:2 * F], xt[:], start=False, stop=True)
                    nc.tensor.matmul(hp[:], gks[:, 2 * F:3 * F], at_sl, start=True, stop=True)
                    nc.tensor.matmul(gq[:], gr[:, 2 * F:3 * F], xt[:], start=True, stop=True)
                    z = p2.tile([128, w], f16, tag="z")
                    r = p2.tile([128, w], f16, tag="r")
                    nc.scalar.activation(z[:], zp[:], AF.Sigmoid, bias=gbias[0][:])
                    nc.scalar.activation(r[:], rp[:], AF.Sigmoid, bias=gbias[1][:])
                    rhb = p2.tile([128, w], f16, tag="rhb")
                    nc.any.tensor_scalar(out=rhb[:], in0=gq[:],
                                         scalar1=ms[:, 4:5], scalar2=None,
                                         op0=ALU.add)
                    t1 = p2.tile([128, w], f16, tag="t1")
                    nc.any.tensor_tensor(out=t1[:], in0=r[:], in1=rhb[:],
                                         op=ALU.mult)
                    t2 = p2.tile([128, w], f16, tag="t2")
                    nc.any.tensor_tensor(out=t2[:], in0=hp[:], in1=t1[:],
                                         op=ALU.add)
                    hc = p2.tile([128, w], f16, tag="hc")
                    nc.scalar.activation(hc[:], t2[:], AF.Tanh, bias=gbias[2][:])
                    dd = p2.tile([128, w], f16, tag="dd")
                    nc.any.tensor_tensor(out=dd[:], in0=xt[:], in1=hc[:],
                                         op=ALU.subtract)
                    mm_ = p2.tile([128, w], f16, tag="mm")
                    nc.any.tensor_tensor(out=mm_[:], in0=z[:], in1=dd[:],
                                         op=ALU.mult)
                    oo = p2.tile([128, w], f16, tag="oo")
                    nc.any.tensor_tensor(out=oo[:], in0=hc[:], in1=mm_[:],
                                         op=ALU.add)
                    nc.sync.dma_start(out_d[:, off:off + w], oo[:])

    return nc


# ---------------------------------------------------------------- kernel

def _prepare(inputs):
    x = np.asarray(inputs["x"], dtype=np.float32)
    n_nodes = x.shape[0]
    ncn = n_nodes // NCORE
    nst = (ncn + ST - 1) // ST
    widths = [ST] * (nst - 1) + [ncn - (nst - 1) * ST]

    pbs_p, npr_p, nblkp, idxp_c, trp_c = _prep_edges(
        inputs["pairs_prev"], n_nodes, ncn, nst)
    pbs_n, npr_n, nblkn, idxn_c, trn_c = _prep_edges(
        inputs["pairs_next"], n_nodes, ncn, nst)

    xf = np.ascontiguousarray(x)
    wn = np.ascontiguousarray(np.asarray(inputs["w_next"], np.float16))
    wpv = np.ascontiguousarray(np.asarray(inputs["w_prev"], np.float16))
    gkv = np.ascontiguousarray(np.asarray(inputs["gru_kernel"], np.float16))
    grv = np.ascontiguousarray(
        np.asarray(inputs["gru_rec_kernel"], np.float16))
    gb = np.asarray(inputs["gru_bias"], dtype=np.float32)
    bb = np.asarray(inputs["b"], dtype=np.float32).reshape(-1)
    gamma = np.asarray(inputs["bn_gamma"], dtype=np.float32).reshape(-1)
    beta = np.asarray(inputs["bn_beta"], dtype=np.float32).reshape(-1)

    misc = np.zeros((128, 8), dtype=np.float32)
    misc[:, 0] = bb
    misc[:, 1] = gb[0, 0:F] + gb[1, 0:F]          # z bias
    misc[:, 2] = gb[0, F:2 * F] + gb[1, F:2 * F]  # r bias
    misc[:, 3] = gb[0, 2 * F:3 * F]               # h kernel bias
    misc[:, 4] = gb[1, 2 * F:3 * F]               # h recurrent bias
    misc[:, 5] = gamma
    misc[:, 6] = beta
    misc[:, 7] = BN_EPS
    jt = np.ascontiguousarray(
        np.broadcast_to(np.arange(ST, dtype=np.float32), (128, ST)))
    ident = np.eye(F, dtype=np.float16)

    in_maps = []
    for c in range(NCORE):
        xt_c = np.ascontiguousarray(
            x[c * ncn:(c + 1) * ncn].T.astype(np.float16))
        in_maps.append({
            "x_full": xf,
            "xtb": xt_c,
            "idx_prev": idxp_c[c], "idx_next": idxn_c[c],
            "trel_prev": trp_c[c], "trel_next": trn_c[c],
            "w_prev": wpv, "w_next": wn,
            "gru_kernel": gkv, "gru_rec": grv,
            "jtile": jt, "ident": ident, "misc": misc,
        })
    nc = _build_program(n_nodes, pbs_p, npr_p, nblkp, pbs_n, npr_n, nblkn, widths)
    return nc, in_maps, ncn


def kernel(**inputs):
    _install_ntff_hook()
    from concourse.bass_utils import run_bass_kernel_spmd
    nc, in_maps, ncn = _prepare(inputs)
    _split_excess_waits(nc, cap=1)
    trace = bool(int(os.environ.get("KERNEL_TRACE", "0")))
    kw = {}
    if trace:
        kw = dict(trace=True,
                  tmpdir=os.environ.get("KERNEL_TRACE_DIR",
                                        "/tmp/kernel_trace"))
    res = run_bass_kernel_spmd(nc, in_maps, list(range(NCORE)), **kw)
    if trace:
        kernel.last_exec_time_ns = res.exec_time_ns
    out = np.concatenate(
        [res.results[c]["outT"].T.astype(np.float32) for c in range(NCORE)],
        axis=0)
    return out


kernel.last_exec_time_ns = None
